# revision 11
# baseline (speedup 1.0000x reference)
"""TRN2 Bass kernel for nn_FAquantizer (vq_codebook).

Data-parallel over batch B=16 across 8 NeuronCores (2 batches/core).

Numerics strategy (validated against the fp32 jax reference):
- WaveNet convs / residual-skip / projections / style encoder: 3-pass
  split-FP22 matmuls (float32r, round-to-nearest-even @ 11 mantissa bits)
  -> fp32-grade accuracy at 1 cycle/row per pass.
- VQ score path (z_e bases, cosine scores): true fp32 matmuls (bit-exact
  operands).
- RVQ residual chain: algebraic rewrite. z_e for later quantizers =
  base matmul vs the branch input + small 8-dim gathered corrections
  from host-precomputed tables; z_q outputs are row gathers from
  host-precomputed (codebook @ out_w.T + out_b) tables.
- Losses: commitment == codebook loss (forward); single SSE accumulator.
- Final AdaIN-LN computed in (t, d) layout; host transposes outputs.
"""
import numpy as np
from contextlib import ExitStack

import concourse.bass as bass
import concourse.bacc as bacc
import concourse.mybir as mybir
import concourse.tile as tile
from concourse.bass import ts
from concourse.bass_utils import run_bass_kernel_spmd

F32 = mybir.dt.float32
F32R = mybir.dt.float32r
U32 = mybir.dt.uint32
AF = mybir.ActivationFunctionType
ALU = mybir.AluOpType

B, T, D, H = 16, 1024, 1024, 256
CB_K, CB_D = 1024, 8
NCORES = 8
BL = B // NCORES          # batches per core = 2
NT = T // 128             # 8 t-tiles
ND = D // 128             # 8 d-tiles
NL_P, NL_C = 8, 16        # wavenet layers

# quantizer graph: global order p0, c0, c1, r0, r1, r2
SUCC = {0: [3, 4, 5], 1: [2, 3, 4, 5], 2: [3, 4, 5], 3: [4, 5], 4: [5], 5: []}
PRED = {0: [], 1: [], 2: [1], 3: [0, 1, 2], 4: [0, 1, 2, 3], 5: [0, 1, 2, 3, 4]}
SMALL_W = {q: 8 * len(SUCC[q]) for q in range(5)}

# tunables
WNW_BUFS = 2
XSP_BUFS = 9


def r22(x):
    """float32r rounding: round-to-nearest-even at 11 mantissa bits."""
    x = np.asarray(x, np.float32)
    xa = np.abs(x).view(np.int32).astype(np.int64)
    step = 1 << 12
    q, rem = np.divmod(xa, step)
    up = (rem > step // 2) | ((rem == step // 2) & ((q & 1) == 1))
    r = ((q + up) * step).astype(np.int64)
    out = r.astype(np.int32).view(np.float32).copy()
    return (out * np.where(np.signbit(x), np.float32(-1), np.float32(1))).astype(np.float32)


def split22(x):
    hi = r22(x)
    lo = r22((np.asarray(x, np.float32) - hi).astype(np.float32))
    return hi, lo


def _blocks_flat(wT, nk=None):
    """wT: (K, M) lhsT -> hi/lo split, tiled into (P, nblk*128) flat layout.

    Returns array (P, 2 * nKblk * nMblk * 128) with block order (s, kb, mb),
    P = min(K, 128) partition rows per block... K must be multiple of 128 or
    <=128. Here handled by caller; this helper is for K multiple of 128.
    """
    raise NotImplementedError


def prep_conv_blocks(w_list, P):
    """w_list: list over block-cols of (P, 128) fp32 arrays (already split).
    -> (P, len*128) flat."""
    return np.concatenate(w_list, axis=1).astype(np.float32)


def host_prepare(inputs):
    """Precompute all device-side constant tensors (shared across cores)."""
    f32 = lambda k: np.ascontiguousarray(np.asarray(inputs[k], np.float32))
    prep = {}

    def conv_w_layout(in_w, in_b):
        """in_w (L, 2H, H, 5) -> (L, 2, 128, 40*128) f32r-split block layout,
        in_b -> (L, 128, 4)."""
        L = in_w.shape[0]
        out = np.zeros((L, 2, 128, 40 * 128), np.float32)
        for l in range(L):
            for k in range(5):
                wT = in_w[l, :, :, k].T.astype(np.float32)   # (256, 512)
                hi, lo = split22(wT)
                for mh in range(2):
                    for s, w_s in enumerate((hi, lo)):
                        for c in range(2):
                            for m2 in range(2):
                                m = mh * 2 + m2
                                blk = ((s * 5 + k) * 2 + c) * 2 + m2
                                out[l, mh, :, blk * 128:(blk + 1) * 128] = \
                                    w_s[c * 128:(c + 1) * 128, m * 128:(m + 1) * 128]
        bias = np.zeros((L, 128, 4), np.float32)
        for l in range(L):
            bias[l] = in_b[l].reshape(4, 128).T
        return out, bias

    def rs_w_layout(rs_w, rs_b):
        """rs_w (L, 2H, H) -> (L, 128, 16*128); rs_b -> (L, 128, 4)."""
        L = rs_w.shape[0]
        out = np.zeros((L, 128, 16 * 128), np.float32)
        for l in range(L):
            hi, lo = split22(rs_w[l].T.astype(np.float32))  # (256, 512)
            for s, w_s in enumerate((hi, lo)):
                for c in range(2):
                    for m in range(4):
                        blk = (s * 2 + c) * 4 + m
                        out[l, :, blk * 128:(blk + 1) * 128] = \
                            w_s[c * 128:(c + 1) * 128, m * 128:(m + 1) * 128]
        bias = np.zeros((L, 128, 4), np.float32)
        for l in range(L):
            bias[l] = rs_b[l].reshape(4, 128).T
        return out, bias

    def proj_layout(w):
        """w (M, K) -> lhsT (K, M) split into (K<=128 rows, 2*nc*nm*128)."""
        wT = w.T.astype(np.float32)  # (K, M)
        K, M = wT.shape
        hi, lo = split22(wT)
        ncb = max(1, K // 128)
        nmb = M // 128
        P = min(K, 128)
        blks = []
        for s, w_s in enumerate((hi, lo)):
            for c in range(ncb):
                for m in range(nmb):
                    blks.append(w_s[c * 128:c * 128 + P, m * 128:(m + 1) * 128])
        return np.concatenate(blks, axis=1).astype(np.float32)

    def bias_cols(b):
        """(M,) -> (128, M/128)"""
        return np.ascontiguousarray(b.reshape(-1, 128).T.astype(np.float32))

    # wavenet weights (prosody 8 + content 16 concatenated)
    mw, mb = conv_w_layout(f32('mwn_in_w'), f32('mwn_in_b'))
    cw, cb_ = conv_w_layout(f32('cwn_in_w'), f32('cwn_in_b'))
    prep['wn_w'] = np.concatenate([mw, cw], 0)
    prep['wn_b'] = np.concatenate([mb, cb_], 0)
    mrw, mrb = rs_w_layout(f32('mwn_rs_w'), f32('mwn_rs_b'))
    crw, crb = rs_w_layout(f32('cwn_rs_w'), f32('cwn_rs_b'))
    prep['rs_w'] = np.concatenate([mrw, crw], 0)
    prep['rs_b'] = np.concatenate([mrb, crb], 0)

    prep['ml_w'] = proj_layout(f32('ml_w'))          # (20, 2*1*2*128=512)
    prep['ml_b'] = bias_cols(f32('ml_b'))            # (128, 2)
    W_mc = (np.float64(f32('cl_w')) @ np.float64(f32('dct_mat')).T).astype(np.float32)
    prep['W_mc'] = proj_layout(W_mc)                 # (80, 512)
    prep['cl_b'] = bias_cols(f32('cl_b'))
    prep['ml2_w'] = proj_layout(f32('ml2_w'))        # (128, 2*2*8*128=4096)
    prep['ml2_b'] = bias_cols(f32('ml2_b'))          # (128, 8)
    prep['cl2_w'] = proj_layout(f32('cl2_w'))
    prep['cl2_b'] = bias_cols(f32('cl2_b'))

    # style encoder
    se1 = np.zeros((80, 20 * 128), np.float32)
    for k in range(5):
        hi, lo = split22(f32('se_w1')[:, :, k].T)    # (80, 256)
        for s, w_s in enumerate((hi, lo)):
            for m in range(2):
                blk = (s * 5 + k) * 2 + m
                se1[:, blk * 128:(blk + 1) * 128] = w_s[:, m * 128:(m + 1) * 128]
    prep['se_w1'] = se1
    prep['se_b1'] = bias_cols(f32('se_b1'))
    se2 = np.zeros((128, 40 * 128), np.float32)
    for k in range(5):
        hi, lo = split22(f32('se_w2')[:, :, k].T)    # (256, 256)
        for s, w_s in enumerate((hi, lo)):
            for c in range(2):
                for m in range(2):
                    blk = ((s * 5 + k) * 2 + c) * 2 + m
                    se2[:, blk * 128:(blk + 1) * 128] = \
                        w_s[c * 128:(c + 1) * 128, m * 128:(m + 1) * 128]
    prep['se_w2'] = se2
    prep['se_b2'] = bias_cols(f32('se_b2'))
    prep['se_wo'] = proj_layout(f32('se_wo'))        # (128, 2*2*8*128)
    prep['se_bo'] = bias_cols(f32('se_bo'))
    Wst = (np.float64(f32('tl_w')) @ np.float64(f32('se_wo'))).astype(np.float32)
    bst = (np.float64(f32('tl_b')) + np.float64(f32('tl_w')) @ np.float64(f32('se_bo'))).astype(np.float32)
    prep['Wst'] = proj_layout(Wst)                   # (128, 2*2*16*128=8192)
    prep['bst'] = bias_cols(bst)                     # (128, 16)

    # RVQ tables
    groups = [('p', 0), ('c', 0), ('c', 1), ('r', 0), ('r', 1), ('r', 2)]
    in_ws, in_bs, cbs, out_rows64 = [], [], [], []
    for g, i in groups:
        in_ws.append(f32(f'{g}_in_w')[i])
        in_bs.append(f32(f'{g}_in_b')[i])
        cbs.append(f32(f'{g}_cb')[i])
        o64 = np.float64(cbs[-1]) @ np.float64(f32(f'{g}_out_w')[i]).T \
            + np.float64(f32(f'{g}_out_b')[i])[None, :]
        out_rows64.append(o64)

    zin = np.zeros((128, 6 * 8 * 8), np.float32)
    for q in range(6):
        wT = in_ws[q].T.astype(np.float32)           # (1024, 8)
        for dt in range(ND):
            zin[:, (q * 8 + dt) * 8:(q * 8 + dt + 1) * 8] = wT[dt * 128:(dt + 1) * 128]
    prep['zin_w'] = zin
    zb = np.zeros((128, 48), np.float32)
    for q in range(6):
        zb[:, q * 8:(q + 1) * 8] = in_bs[q][None, :]
    prep['zin_b'] = zb

    cbn = np.zeros((6, 8, 1024), np.float32)
    for q in range(6):
        n = np.maximum(np.linalg.norm(cbs[q], axis=-1, keepdims=True),
                       np.float32(1e-12)).astype(np.float32)
        cbn[q] = (cbs[q] / n).T
    prep['cbn'] = cbn

    for q in range(6):
        big = np.zeros((CB_K, 1032), np.float32)
        big[:, :1024] = out_rows64[q].astype(np.float32)
        big[:, 1024:] = cbs[q]
        prep[f'big{q}'] = big
    for q in range(5):
        sm = np.zeros((CB_K, SMALL_W[q]), np.float32)
        for j, s in enumerate(SUCC[q]):
            sm[:, 8 * j:8 * (j + 1)] = (out_rows64[q] @ np.float64(in_ws[s]).T).astype(np.float32)
        prep[f'small{q}'] = sm

    prep['ident'] = np.eye(128, dtype=np.float32)
    return prep


def build_nc():
    nc = bacc.Bacc("TRN2", debug=False, num_devices=NCORES)
    P = {}

    def par(name, shape, dtype=F32, out=False):
        P[name] = nc.declare_dram_parameter(name, list(shape), dtype, isOutput=out)
        return P[name]

    # inputs (per-core slices)
    par('x', (BL, D, T))
    par('prosody', (BL, 20, T))
    par('mel', (BL, 80, T))
    par('maskp', (BL, 128, 1))
    # weights
    par('wn_w', (24, 2, 128, 40 * 128), F32R)
    par('wn_b', (24, 128, 4))
    par('rs_w', (24, 128, 16 * 128), F32R)
    par('rs_b', (24, 128, 4))
    par('ml_w', (20, 512), F32R); par('ml_b', (128, 2))
    par('W_mc', (80, 512), F32R); par('cl_b', (128, 2))
    par('ml2_w', (128, 4096), F32R); par('ml2_b', (128, 8))
    par('cl2_w', (128, 4096), F32R); par('cl2_b', (128, 8))
    par('se_w1', (80, 2560), F32R); par('se_b1', (128, 2))
    par('se_w2', (128, 5120), F32R); par('se_b2', (128, 2))
    par('se_wo', (128, 4096), F32R); par('se_bo', (128, 8))
    par('Wst', (128, 8192), F32R); par('bst', (128, 16))
    par('zin_w', (128, 384)); par('zin_b', (128, 48))
    par('cbn', (6, 8, 1024))
    for q in range(6):
        par(f'big{q}', (CB_K, 1032))
    for q in range(5):
        par(f'small{q}', (CB_K, SMALL_W[q]))
    par('ident', (128, 128))
    # outputs
    par('o_td', (BL, T, D), out=True)
    par('zp_td', (BL, T, D), out=True)
    par('zc_td', (BL, T, D), out=True)
    par('zr_td', (BL, T, D), out=True)
    par('timbre_o', (BL, 1024), out=True)
    par('sse_o', (128, 1), out=True)

    ap = lambda n: P[n].ap()

    with tile.TileContext(nc) as tc, ExitStack() as ctx:
        # ---- persistent pools ----
        const = ctx.enter_context(tc.tile_pool(name="const", bufs=1))
        psA = ctx.enter_context(tc.tile_pool(name="psA", bufs=2, space="PSUM"))   # conv, rs
        psB = ctx.enter_context(tc.tile_pool(name="psB", bufs=2, space="PSUM"))   # zb
        psC = ctx.enter_context(tc.tile_pool(name="psC", bufs=1, space="PSUM"))   # tp
        psD = ctx.enter_context(tc.tile_pool(name="psD", bufs=1, space="PSUM"))   # sc
        dram = ctx.enter_context(tc.tile_pool(name="dram", bufs=1, space="DRAM"))
        persist = ctx.enter_context(tc.tile_pool(name="persist", bufs=1))

        identt = const.tile([128, 128], F32)
        nc.sync.dma_start(identt[:], ap('ident'))
        zinw = const.tile([128, 384], F32)
        nc.sync.dma_start(zinw[:], ap('zin_w'))
        zinb = const.tile([128, 48], F32)
        nc.sync.dma_start(zinb[:], ap('zin_b'))
        maskt = const.tile([128, BL], F32)
        for b in range(BL):
            nc.sync.dma_start(maskt[:, b:b + 1], ap('maskp')[b])

        zeT = persist.tile([128, 6 * BL * NT * 8], F32, name="zeT")       # 3KB/p
        idxT = persist.tile([128, 6 * BL * NT], U32, name="idxT")
        sseT = persist.tile([128, 6 * BL * NT], F32, name="sseT")
        smal = persist.tile([128, 5 * BL * NT * 32], F32, name="smal")     # corr gathers

        chn = ctx.enter_context(tc.tile_pool(name="chn", bufs=2))
        cbn_pool = ctx.enter_context(tc.tile_pool(name="cbnp", bufs=1))

        zcol = lambda q, b, t: ((q * BL + b) * NT + t) * 8
        icol = lambda q, b, t: (q * BL + b) * NT + t
        scol = lambda q, b, t: ((q * BL + b) * NT + t) * 32

        # style row scratch in DRAM
        st_scr = [dram.tile([1, 2048], F32, name=f"stscr{b}", tag=f"stscr{b}") for b in range(BL)]
        # f0/cont scratch
        br_scr = {}
        for b in range(BL):
            for m in range(ND):
                br_scr[(b, m)] = dram.tile([128, T], F32, name=f"brscr{b}_{m}", tag=f"brscr{b}_{m}")

        # ================= helpers =================
        def mm3_list(psum, triples, n=512):
            """triples: list of (whi, wlo, xr, xlo) matmuls accumulated; each
            expands to 3 passes."""
            mms = []
            for whi, wlo, xr, xlo in triples:
                mms.append((whi, xr))
                mms.append((whi, xlo))
                mms.append((wlo, xr))
            for i, (lh, rh) in enumerate(mms):
                nc.tensor.matmul(psum, lh, rh, start=(i == 0), stop=(i == len(mms) - 1))

        def split_dev(pool, src_ap, shape, tag):
            """returns (hi_f32r_tile, lo_f32r_tile) of src_ap."""
            hi = pool.tile(shape, F32R, name=f"hi_{tag}", tag="xsp")
            lo = pool.tile(shape, F32R, name=f"lo_{tag}", tag="xsp")
            nc.vector.tensor_copy(hi[:], src_ap)
            nc.vector.tensor_sub(lo[:], src_ap, hi[:].bitcast(F32))
            return hi, lo

        # ================= P1: style encoder =================
        with tc.tile_pool(name="p1", bufs=1) as p1, \
             tc.tile_pool(name="melp", bufs=1) as melp:
            se1t = p1.tile([80, 2560], F32R)
            nc.sync.dma_start(se1t[:], ap('se_w1'))
            se2t = p1.tile([128, 5120], F32R)
            nc.sync.dma_start(se2t[:], ap('se_w2'))
            sewot = p1.tile([128, 4096], F32R)
            nc.sync.dma_start(sewot[:], ap('se_wo'))
            wstt = p1.tile([128, 8192], F32R)
            nc.sync.dma_start(wstt[:], ap('Wst'))
            seb1 = p1.tile([128, 2], F32); nc.sync.dma_start(seb1[:], ap('se_b1'))
            seb2 = p1.tile([128, 2], F32); nc.sync.dma_start(seb2[:], ap('se_b2'))
            sebo = p1.tile([128, 8], F32); nc.sync.dma_start(sebo[:], ap('se_bo'))
            bstt = p1.tile([128, 16], F32); nc.sync.dma_start(bstt[:], ap('bst'))

            melt, melr, mello = {}, {}, {}
            for b in range(BL):
                mt = melp.tile([80, T + 4], F32, name=f"mel{b}", tag=f"mel{b}")
                nc.gpsimd.memset(mt[:, 0:2], 0.0)
                nc.gpsimd.memset(mt[:, T + 2:T + 4], 0.0)
                nc.sync.dma_start(mt[:, 2:T + 2], ap('mel')[b])
                mr = melp.tile([80, T + 4], F32R, name=f"melr{b}", tag=f"melr{b}")
                ml = melp.tile([80, T + 4], F32R, name=f"mello{b}", tag=f"mello{b}")
                nc.vector.tensor_copy(mr[:], mt[:])
                nc.vector.tensor_sub(ml[:], mt[:], mr[:].bitcast(F32))
                melt[b], melr[b], mello[b] = mt, mr, ml

            h1, h1r, h1lo = {}, {}, {}
            h2 = {}
            for b in range(BL):
                for m in range(2):
                    t_ = p1.tile([128, T + 4], F32, name=f"h1_{b}_{m}", tag=f"h1_{b}_{m}")
                    nc.gpsimd.memset(t_[:, 0:2], 0.0)
                    nc.gpsimd.memset(t_[:, T + 2:T + 4], 0.0)
                    h1[(b, m)] = t_
            # conv1 (contract 80)
            blk1 = lambda s, k, m: (s * 5 + k) * 2 + m
            for b in range(BL):
                for m in range(2):
                    for n in range(2):
                        ps = psA.tile([128, 512], F32, tag="conv")
                        mms = []
                        for s_w, x_t in ((0, melr[b]), (0, mello[b]), (1, melr[b])):
                            for k in range(5):
                                mms.append((se1t[:, ts(blk1(s_w, k, m), 128)],
                                            x_t[:, n * 512 + k: n * 512 + k + 512]))
                        for i, (lh, rh) in enumerate(mms):
                            nc.tensor.matmul(ps[:], lh, rh, start=(i == 0), stop=(i == len(mms) - 1))
                        # bias + leaky relu 0.2 into padded h1
                        dst = h1[(b, m)][:, 2 + n * 512: 2 + n * 512 + 512]
                        nc.scalar.activation(dst, ps[:], AF.Identity, bias=seb1[:, m:m + 1])
                        nc.vector.scalar_tensor_tensor(
                            out=dst, in0=dst, scalar=0.2, in1=dst,
                            op0=ALU.mult, op1=ALU.max)
            for b in range(BL):
                for m in range(2):
                    r_ = p1.tile([128, T + 4], F32R, name=f"h1r_{b}_{m}", tag=f"h1r_{b}_{m}")
                    l_ = p1.tile([128, T + 4], F32R, name=f"h1l_{b}_{m}", tag=f"h1l_{b}_{m}")
                    nc.vector.tensor_copy(r_[:], h1[(b, m)][:])
                    nc.vector.tensor_sub(l_[:], h1[(b, m)][:], r_[:].bitcast(F32))
                    h1r[(b, m)], h1lo[(b, m)] = r_, l_
            # conv2 (contract 256)
            blk2 = lambda s, k, c, m: ((s * 5 + k) * 2 + c) * 2 + m
            for b in range(BL):
                for m in range(2):
                    h2[(b, m)] = p1.tile([128, T], F32, name=f"h2_{b}_{m}", tag=f"h2_{b}_{m}")
                    for n in range(2):
                        ps = psA.tile([128, 512], F32, tag="conv")
                        mms = []
                        for s_w, hr, hl in ((0, h1r, None), (0, None, h1lo), (1, h1r, None)):
                            for k in range(5):
                                for c in range(2):
                                    x_t = hr[(b, c)] if hr is not None else hl[(b, c)]
                                    mms.append((se2t[:, ts(blk2(s_w, k, c, m), 128)],
                                                x_t[:, n * 512 + k: n * 512 + k + 512]))
                        for i, (lh, rh) in enumerate(mms):
                            nc.tensor.matmul(ps[:], lh, rh, start=(i == 0), stop=(i == len(mms) - 1))
                        dst = h2[(b, m)][:, ts(n, 512)]
                        nc.scalar.activation(dst, ps[:], AF.Identity, bias=seb2[:, m:m + 1])
                        nc.vector.scalar_tensor_tensor(
                            out=dst, in0=dst, scalar=0.2, in1=dst,
                            op0=ALU.mult, op1=ALU.max)
            # mean over T, then timbre / style matvecs (N=2: both batches)
            hmall = p1.tile([128, 4], F32, name="hmall", tag="hmall")  # col = c*2+b
            for b in range(BL):
                for m in range(2):
                    s_ = p1.tile([128, 1], F32, name=f"hs{b}{m}", tag="hs")
                    nc.vector.reduce_sum(s_[:], h2[(b, m)][:], axis=mybir.AxisListType.X)
                    nc.scalar.mul(hmall[:, m * 2 + b:m * 2 + b + 1], s_[:], 1.0 / T)
            hmr = p1.tile([128, 4], F32R, name="hmr", tag="hmr")
            hmlo = p1.tile([128, 4], F32R, name="hmlo", tag="hmlo")
            nc.vector.tensor_copy(hmr[:], hmall[:])
            nc.vector.tensor_sub(hmlo[:], hmall[:], hmr[:].bitcast(F32))
            pblk = lambda s, c, m, nm: (s * 2 + c) * nm + m
            timball = p1.tile([128, 16], F32, name="timball", tag="timball")  # col m*2+b
            for m in range(8):
                ps = psB.tile([128, 8], F32, tag="zb")
                mms = []
                for s_w, x_t in ((0, hmr), (0, hmlo), (1, hmr)):
                    for c in range(2):
                        mms.append((sewot[:, ts(pblk(s_w, c, m, 8), 128)],
                                    x_t[:, c * 2:c * 2 + 2]))
                for i, (lh, rh) in enumerate(mms):
                    nc.tensor.matmul(ps[:, 0:2], lh, rh, start=(i == 0), stop=(i == len(mms) - 1))
                nc.scalar.activation(timball[:, m * 2:m * 2 + 2], ps[:, 0:2], AF.Identity,
                                     bias=sebo[:, m:m + 1])
            stall = p1.tile([128, 32], F32, name="stall", tag="stall")  # col m*2+b
            for m in range(16):
                ps = psB.tile([128, 8], F32, tag="zb")
                mms = []
                for s_w, x_t in ((0, hmr), (0, hmlo), (1, hmr)):
                    for c in range(2):
                        mms.append((wstt[:, ts(pblk(s_w, c, m, 16), 128)],
                                    x_t[:, c * 2:c * 2 + 2]))
                for i, (lh, rh) in enumerate(mms):
                    nc.tensor.matmul(ps[:, 0:2], lh, rh, start=(i == 0), stop=(i == len(mms) - 1))
                nc.scalar.activation(stall[:, m * 2:m * 2 + 2], ps[:, 0:2], AF.Identity,
                                     bias=bstt[:, m:m + 1])
            for b in range(BL):
                nc.sync.dma_start(
                    ap('timbre_o')[b:b + 1, :].rearrange("o (m p) -> (o p) m", p=128),
                    timball[:, :].rearrange("p (m b) -> p b m", b=2)[:, b])
                nc.sync.dma_start(
                    st_scr[b][0:1, :].rearrange("o (m p) -> (o p) m", p=128),
                    stall[:, :].rearrange("p (m b) -> p b m", b=2)[:, b])

        # ================= WaveNet machinery =================
        wn_stack = ExitStack()
        wn = wn_stack.enter_context(tc.tile_pool(name="wn", bufs=1))
        wwp = wn_stack.enter_context(tc.tile_pool(name="wwp", bufs=WNW_BUFS))
        xsp = wn_stack.enter_context(tc.tile_pool(name="xsp", bufs=XSP_BUFS))

        xw = {}
        ow = {}
        for b in range(BL):
            for c2 in range(2):
                t_ = wn.tile([128, T + 4], F32, name=f"xw{b}{c2}", tag=f"xw{b}{c2}")
                nc.gpsimd.memset(t_[:, 0:2], 0.0)
                nc.gpsimd.memset(t_[:, T + 2:T + 4], 0.0)
                xw[(b, c2)] = t_
                ow[(b, c2)] = wn.tile([128, T], F32, name=f"ow{b}{c2}", tag=f"ow{b}{c2}")

        cblk = lambda s, k, c, m2: ((s * 5 + k) * 2 + c) * 2 + m2
        rblk = lambda s, c, m: (s * 2 + c) * 4 + m

        def wavenet_layer(l, last):
            wc = [wwp.tile([128, 40 * 128], F32R, name=f"wc{l}_{mh}", tag="wnw", bufs=3)
                  for mh in range(2)]
            nc.sync.dma_start(wc[0][:], ap('wn_w')[l, 0])
            nc.sync.dma_start(wc[1][:], ap('wn_w')[l, 1])
            rc = wwp.tile([128, 16 * 128], F32R, name=f"rc{l}", tag="rsw", bufs=1)
            nc.sync.dma_start(rc[:], ap('rs_w')[l])
            wb = wwp.tile([128, 4], F32, name=f"wb{l}", tag="wnb")
            nc.sync.dma_start(wb[:], ap('wn_b')[l])
            rb = wwp.tile([128, 4], F32, name=f"rb{l}", tag="rsb")
            nc.sync.dma_start(rb[:], ap('rs_b')[l])

            xr, xlo = {}, {}
            for b in range(BL):
                for c in range(2):
                    xr[(b, c)], xlo[(b, c)] = split_dev(xsp, xw[(b, c)][:], [128, T + 4], f"x{l}{b}{c}")
            sg = {}
            acts = {}
            for b in range(BL):
                for c in range(2):
                    sg[(b, c)] = xsp.tile([128, T], F32, name=f"sg{l}{b}{c}", tag="xsp")
                    acts[(b, c)] = xsp.tile([128, T], F32, name=f"ac{l}{b}{c}", tag="acts", bufs=4)
            for mh in range(2):
                for b in range(BL):
                    for m2 in range(2):
                        m = mh * 2 + m2
                        for n in range(2):
                            ps = psA.tile([128, 512], F32, tag="conv")
                            mms = []
                            for s_w, xd in ((0, xr), (0, xlo), (1, xr)):
                                for k in range(5):
                                    for c in range(2):
                                        mms.append((wc[mh][:, ts(cblk(s_w, k, c, m2), 128)],
                                                    xd[(b, c)][:, n * 512 + k:n * 512 + k + 512]))
                            for i, (lh, rh) in enumerate(mms):
                                nc.tensor.matmul(ps[:], lh, rh, start=(i == 0), stop=(i == len(mms) - 1))
                            if m < 2:
                                nc.scalar.activation(acts[(b, m)][:, ts(n, 512)], ps[:],
                                                     AF.Tanh, bias=wb[:, m:m + 1])
                            else:
                                nc.scalar.activation(sg[(b, m - 2)][:, ts(n, 512)], ps[:],
                                                     AF.Sigmoid, bias=wb[:, m:m + 1])
            ar, alo = {}, {}
            for b in range(BL):
                for c in range(2):
                    nc.vector.tensor_mul(acts[(b, c)][:], acts[(b, c)][:], sg[(b, c)][:])
                    ar[(b, c)], alo[(b, c)] = split_dev(xsp, acts[(b, c)][:], [128, T], f"a{l}{b}{c}")
            for b in range(BL):
                for m in range(4):
                    for n in range(2):
                        if m < 2 and last:
                            continue
                        ps = psA.tile([128, 512], F32, tag="rs")
                        mms = []
                        for s_w, ad in ((0, ar), (0, alo), (1, ar)):
                            for c in range(2):
                                mms.append((rc[:, ts(rblk(s_w, c, m), 128)],
                                            ad[(b, c)][:, ts(n, 512)]))
                        for i, (lh, rh) in enumerate(mms):
                            nc.tensor.matmul(ps[:], lh, rh, start=(i == 0), stop=(i == len(mms) - 1))
                        if m < 2:
                            dst = xw[(b, m)][:, 2 + n * 512: 2 + n * 512 + 512]
                        else:
                            dst = ow[(b, m - 2)][:, ts(n, 512)]
                        nc.vector.scalar_tensor_tensor(
                            out=dst, in0=ps[:], scalar=rb[:, m:m + 1], in1=dst,
                            op0=ALU.add, op1=ALU.add)

        def branch_proj(wname, bname, dst_scr):
            """project ow (2H=... here h=256 -> 1024) via 1024x256 matrix."""
            wt = wn.tile([128, 4096], F32R, name=f"proj_{wname}", tag="projw")
            nc.sync.dma_start(wt[:], ap(wname))
            bt = wn.tile([128, 8], F32, name=f"projb_{bname}", tag="projb")
            nc.sync.dma_start(bt[:], ap(bname))
            orr, olo = {}, {}
            for b in range(BL):
                for c in range(2):
                    orr[(b, c)], olo[(b, c)] = split_dev(xsp, ow[(b, c)][:], [128, T], f"o{wname}{b}{c}")
            pb = lambda s, c, m: (s * 2 + c) * 8 + m
            for b in range(BL):
                for m in range(8):
                    ft = xsp.tile([128, T], F32, name=f"ft{wname}{b}{m}", tag="ftmp", bufs=2)
                    for n in range(2):
                        ps = psA.tile([128, 512], F32, tag="conv")
                        mms = []
                        for s_w, od in ((0, orr), (0, olo), (1, orr)):
                            for c in range(2):
                                mms.append((wt[:, ts(pb(s_w, c, m), 128)],
                                            od[(b, c)][:, ts(n, 512)]))
                        for i, (lh, rh) in enumerate(mms):
                            nc.tensor.matmul(ps[:], lh, rh, start=(i == 0), stop=(i == len(mms) - 1))
                        nc.scalar.activation(ft[:, ts(n, 512)], ps[:], AF.Identity,
                                             bias=bt[:, m:m + 1])
                    nc.sync.dma_start(dst_scr[(b, m)][:], ft[:])

        # ---- chain helpers ----
        def zbase(qlist, src_of_bt):
            """compute z_eT bases for quantizers qlist; src_of_bt(b, dt, tt) -> lhsT AP."""
            for b in range(BL):
                for tt in range(NT):
                    blks = []
                    for dt in range(ND):
                        bl_ = chn.tile([128, 128], F32, name=f"zb{qlist[0]}{b}{tt}{dt}", tag="zblk", bufs=10)
                        nc.sync.dma_start(bl_[:], src_of_bt(b, dt, tt))
                        blks.append(bl_)
                    for q in qlist:
                        ps = psB.tile([128, 8], F32, tag="zb")
                        for dt in range(ND):
                            nc.tensor.matmul(ps[:], blks[dt][:],
                                             zinw[:, ts(q * 8 + dt, 8)],
                                             start=(dt == 0), stop=(dt == ND - 1))
                        nc.vector.tensor_add(zeT[:, ts(zcol(q, b, tt) // 8, 8)],
                                             ps[:], zinb[:, ts(q, 8)])

        def chain_q(q, cbnt):
            """corrections, scores, argmax, small-gather for quantizer q."""
            for b in range(BL):
                for tt in range(NT):
                    zslice = zeT[:, ts(zcol(q, b, tt) // 8, 8)]
                    for pq in PRED[q]:
                        off = 8 * SUCC[pq].index(q)
                        nc.vector.tensor_sub(
                            zslice, zslice,
                            smal[:, scol(pq, b, tt) + off: scol(pq, b, tt) + off + 8])
                    # transpose to (8, t)
                    pt = psC.tile([8, 128], F32, tag="tp")
                    nc.tensor.transpose(pt[:], zslice, identt[:])
                    zet = chn.tile([8, 128], F32, name=f"zet{q}{b}{tt}", tag="zes", bufs=2)
                    nc.vector.tensor_copy(zet[:], pt[:])
                    sc = chn.tile([128, 1024], F32, name=f"sc{q}{b}{tt}", tag="scsb", bufs=1)
                    for hf in range(2):
                        psc = psD.tile([128, 512], F32, tag="sc")
                        nc.tensor.matmul(psc[:], zet[:], cbnt[:, ts(hf, 512)],
                                         start=True, stop=True)
                        nc.vector.tensor_copy(sc[:, ts(hf, 512)], psc[:])
                    mx = chn.tile([128, 8], F32, name=f"mx{q}{b}{tt}", tag="mx", bufs=2)
                    ix = chn.tile([128, 8], U32, name=f"ix{q}{b}{tt}", tag="ix", bufs=2)
                    nc.vector.max(mx[:], sc[:])
                    nc.vector.max_index(ix[:], mx[:], sc[:])
                    nc.vector.tensor_copy(idxT[:, icol(q, b, tt):icol(q, b, tt) + 1],
                                          ix[:, 0:1])
                    if q < 5:
                        w = SMALL_W[q]
                        nc.gpsimd.indirect_dma_start(
                            out=smal[:, scol(q, b, tt):scol(q, b, tt) + w],
                            out_offset=None,
                            in_=ap(f'small{q}'),
                            in_offset=bass.IndirectOffsetOnAxis(
                                ap=idxT[:, icol(q, b, tt):icol(q, b, tt) + 1], axis=0))

        def load_cbn(q):
            t_ = cbn_pool.tile([8, 1024], F32, name=f"cbn{q}", tag="cbn")
            nc.sync.dma_start(t_[:], ap('cbn')[q])
            return t_

        # ================= P2: prosody =================
        if True:
            mlwt = xsp.tile([20, 512], F32R, name="mlwt", tag="xsp")
            nc.sync.dma_start(mlwt[:], ap('ml_w'))
            mlbt = chn.tile([128, 2], F32, name="mlbt", tag="mx", bufs=2)
            nc.sync.dma_start(mlbt[:], ap('ml_b'))
            prt = {}
            for b in range(BL):
                pt_ = xsp.tile([20, T], F32, name=f"pr{b}", tag="xsp")
                nc.sync.dma_start(pt_[:], ap('prosody')[b])
                prr = xsp.tile([20, T], F32R, name=f"prr{b}", tag="xsp")
                prl = xsp.tile([20, T], F32R, name=f"prl{b}", tag="xsp")
                nc.vector.tensor_copy(prr[:], pt_[:])
                nc.vector.tensor_sub(prl[:], pt_[:], prr[:].bitcast(F32))
                prt[b] = (prr, prl)
            for b in range(BL):
                for m in range(2):
                    for n in range(2):
                        ps = psA.tile([128, 512], F32, tag="conv")
                        mms = []
                        for s_w, xi in ((0, 0), (0, 1), (1, 0)):
                            mms.append((mlwt[:, ts(s_w * 2 + m, 128)],
                                        prt[b][xi][:, ts(n, 512)]))
                        for i, (lh, rh) in enumerate(mms):
                            nc.tensor.matmul(ps[:], lh, rh, start=(i == 0), stop=(i == len(mms) - 1))
                        nc.scalar.activation(xw[(b, m)][:, 2 + n * 512:2 + n * 512 + 512],
                                             ps[:], AF.Identity, bias=mlbt[:, m:m + 1])
            for b in range(BL):
                for c in range(2):
                    nc.gpsimd.memset(ow[(b, c)][:], 0.0)
        for l in range(NL_P):
            wavenet_layer(l, last=(l == NL_P - 1))
        branch_proj('ml2_w', 'ml2_b', br_scr)
        zbase([0], lambda b, dt, tt: br_scr[(b, dt)][:, ts(tt, 128)])
        chain_q(0, load_cbn(0))

        # ================= P3: content =================
        if True:
            wmct = xsp.tile([80, 512], F32R, name="wmct", tag="xsp")
            nc.sync.dma_start(wmct[:], ap('W_mc'))
            clbt = chn.tile([128, 2], F32, name="clbt", tag="mx", bufs=2)
            nc.sync.dma_start(clbt[:], ap('cl_b'))
            m3r, m3l = {}, {}
            for b in range(BL):
                m3 = xsp.tile([80, T], F32, name=f"m3{b}", tag="xsp")
                nc.sync.dma_start(m3[:], ap('mel')[b])
                m3r[b] = xsp.tile([80, T], F32R, name=f"m3r{b}", tag="xsp")
                m3l[b] = xsp.tile([80, T], F32R, name=f"m3l{b}", tag="xsp")
                nc.vector.tensor_copy(m3r[b][:], m3[:])
                nc.vector.tensor_sub(m3l[b][:], m3[:], m3r[b][:].bitcast(F32))
            for b in range(BL):
                for m in range(2):
                    for n in range(2):
                        ps = psA.tile([128, 512], F32, tag="conv")
                        mms = []
                        for s_w, x_t in ((0, m3r[b]), (0, m3l[b]), (1, m3r[b])):
                            mms.append((wmct[:, ts(s_w * 2 + m, 128)],
                                        x_t[:, ts(n, 512)]))
                        for i, (lh, rh) in enumerate(mms):
                            nc.tensor.matmul(ps[:], lh, rh, start=(i == 0), stop=(i == len(mms) - 1))
                        nc.scalar.activation(xw[(b, m)][:, 2 + n * 512:2 + n * 512 + 512],
                                             ps[:], AF.Identity, bias=clbt[:, m:m + 1])
            for b in range(BL):
                for c in range(2):
                    nc.gpsimd.memset(ow[(b, c)][:], 0.0)
        for l in range(NL_P, NL_P + NL_C):
            wavenet_layer(l, last=(l == NL_P + NL_C - 1))
        branch_proj('cl2_w', 'cl2_b', br_scr)
        wn_stack.close()
        zbase([1, 2], lambda b, dt, tt: br_scr[(b, dt)][:, ts(tt, 128)])
        chain_q(1, load_cbn(1))
        chain_q(2, load_cbn(2))

        # ================= P4: residual group =================
        zbase([3, 4, 5], lambda b, dt, tt: ap('x')[b, dt * 128:(dt + 1) * 128, ts(tt, 128)])
        chain_q(3, load_cbn(3))
        chain_q(4, load_cbn(4))
        chain_q(5, load_cbn(5))

        # ================= P5: gathers, outputs, LN =================
        with tc.tile_pool(name="p5", bufs=8) as p5g, \
             tc.tile_pool(name="p5w", bufs=4) as p5w, \
             tc.tile_pool(name="p5c", bufs=1) as p5c:
            # gamma/beta broadcast tiles
            gb = {}
            for b in range(BL):
                grow = p5c.tile([1, 1024], F32, name=f"grow{b}", tag=f"grow{b}")
                brow = p5c.tile([1, 1024], F32, name=f"brow{b}", tag=f"brow{b}")
                nc.sync.dma_start(grow[:], st_scr[b][0:1, 0:1024])
                nc.sync.dma_start(brow[:], st_scr[b][0:1, 1024:2048])
                gt = p5c.tile([128, 1024], F32, name=f"gbc{b}", tag=f"gbc{b}")
                bt_ = p5c.tile([128, 1024], F32, name=f"bbc{b}", tag=f"bbc{b}")
                nc.gpsimd.partition_broadcast(gt[:], grow[:])
                nc.gpsimd.partition_broadcast(bt_[:], brow[:])
                gb[b] = (gt, bt_)

            for b in range(BL):
                for tt in range(NT):
                    g = []
                    for q in range(6):
                        gt_ = p5g.tile([128, 1032], F32, name=f"g{q}{b}{tt}", tag="bg", bufs=8)
                        nc.gpsimd.indirect_dma_start(
                            out=gt_[:], out_offset=None, in_=ap(f'big{q}'),
                            in_offset=bass.IndirectOffsetOnAxis(
                                ap=idxT[:, icol(q, b, tt):icol(q, b, tt) + 1], axis=0))
                        g.append(gt_)
                        # loss: (z_eT - cb[idx])^2 summed
                        df = p5w.tile([128, 8], F32, name=f"df{q}{b}{tt}", tag="df", bufs=2)
                        nc.vector.tensor_sub(df[:], zeT[:, ts(zcol(q, b, tt) // 8, 8)],
                                             gt_[:, 1024:1032])
                        dsq = p5w.tile([128, 8], F32, name=f"dsq{q}{b}{tt}", tag="dsq", bufs=2)
                        nc.scalar.activation(dsq[:], df[:], AF.Square,
                                             accum_out=sseT[:, icol(q, b, tt):icol(q, b, tt) + 1])
                    # outputs
                    trow = slice(tt * 128, (tt + 1) * 128)
                    nc.sync.dma_start(ap('zp_td')[b, trow, :], g[0][:, 0:1024])
                    zc_ = p5w.tile([128, 1024], F32, name=f"zc{b}{tt}", tag="zacc", bufs=6)
                    nc.vector.tensor_add(zc_[:], g[1][:, 0:1024], g[2][:, 0:1024])
                    nc.sync.dma_start(ap('zc_td')[b, trow, :], zc_[:])
                    zr_ = p5w.tile([128, 1024], F32, name=f"zr{b}{tt}", tag="zacc", bufs=6)
                    nc.vector.tensor_add(zr_[:], g[3][:, 0:1024], g[4][:, 0:1024])
                    nc.vector.tensor_add(zr_[:], zr_[:], g[5][:, 0:1024])
                    nc.sync.dma_start(ap('zr_td')[b, trow, :], zr_[:])
                    outs_ = p5w.tile([128, 1024], F32, name=f"outs{b}{tt}", tag="zacc", bufs=6)
                    nc.vector.tensor_add(outs_[:], g[0][:, 0:1024], zc_[:])
                    nc.vector.scalar_tensor_tensor(
                        out=outs_[:], in0=zr_[:], scalar=maskt[:, b:b + 1], in1=outs_[:],
                        op0=ALU.mult, op1=ALU.add)
                    # layernorm over free dim
                    s1 = p5w.tile([128, 1], F32, name=f"s1{b}{tt}", tag="ln1", bufs=12)
                    nc.vector.reduce_sum(s1[:], outs_[:], axis=mybir.AxisListType.X)
                    mu = p5w.tile([128, 1], F32, name=f"mu{b}{tt}", tag="ln1", bufs=12)
                    nc.scalar.mul(mu[:], s1[:], 1.0 / D)
                    sqd = p5w.tile([128, 1024], F32, name=f"sqd{b}{tt}", tag="sqd", bufs=2)
                    s2 = p5w.tile([128, 1], F32, name=f"s2{b}{tt}", tag="ln1", bufs=12)
                    nc.scalar.activation(sqd[:], outs_[:], AF.Square, accum_out=s2[:])
                    e2 = p5w.tile([128, 1], F32, name=f"e2{b}{tt}", tag="ln1", bufs=12)
                    nc.scalar.mul(e2[:], s2[:], 1.0 / D)
                    mu2 = p5w.tile([128, 1], F32, name=f"mu2{b}{tt}", tag="ln1", bufs=12)
                    nc.vector.tensor_mul(mu2[:], mu[:], mu[:])
                    var = p5w.tile([128, 1], F32, name=f"var{b}{tt}", tag="ln1", bufs=12)
                    nc.vector.tensor_sub(var[:], e2[:], mu2[:])
                    nc.vector.tensor_scalar(out=var[:], in0=var[:], scalar1=1e-5,
                                            scalar2=None, op0=ALU.add)
                    std = p5w.tile([128, 1], F32, name=f"std{b}{tt}", tag="ln1", bufs=12)
                    nc.scalar.activation(std[:], var[:], AF.Sqrt)
                    rstd = p5w.tile([128, 1], F32, name=f"rstd{b}{tt}", tag="ln1", bufs=12)
                    nc.vector.reciprocal(rstd[:], std[:])
                    nb = p5w.tile([128, 1], F32, name=f"nb{b}{tt}", tag="ln1", bufs=12)
                    nc.vector.tensor_mul(nb[:], mu[:], rstd[:])
                    nc.vector.tensor_scalar(out=nb[:], in0=nb[:], scalar1=-1.0,
                                            scalar2=None, op0=ALU.mult)
                    nt_ = p5w.tile([128, 1024], F32, name=f"nt{b}{tt}", tag="zacc", bufs=6)
                    nc.scalar.activation(nt_[:], outs_[:], AF.Identity,
                                         bias=nb[:], scale=rstd[:])
                    nc.vector.tensor_mul(nt_[:], nt_[:], gb[b][0][:])
                    nc.vector.tensor_add(nt_[:], nt_[:], gb[b][1][:])
                    nc.sync.dma_start(ap('o_td')[b, trow, :], nt_[:])

            # SSE reduce
            sred = p5c.tile([128, 1], F32, name="sred", tag="sred")
            nc.vector.reduce_sum(sred[:], sseT[:], axis=mybir.AxisListType.X)
            nc.sync.dma_start(ap('sse_o'), sred[:])

    nc.compile()
    return nc


_CACHE = {}


def kernel(**inputs):
    if 'nc' not in _CACHE:
        _CACHE['nc'] = build_nc()
    nc = _CACHE['nc']
    prep = host_prepare(inputs)

    x = np.ascontiguousarray(np.asarray(inputs['x'], np.float32))
    pros = np.ascontiguousarray(np.asarray(inputs['prosody_feature'], np.float32))
    mel = np.ascontiguousarray(np.asarray(inputs['mel_segments'], np.float32))
    res_mask = np.asarray(inputs['res_mask'], np.float32).reshape(B)

    in_maps = []
    for ci in range(NCORES):
        sl = slice(ci * BL, (ci + 1) * BL)
        m = dict(prep)
        m['x'] = x[sl]
        m['prosody'] = pros[sl]
        m['mel'] = mel[sl]
        m['maskp'] = np.ascontiguousarray(
            np.broadcast_to(res_mask[sl, None, None], (BL, 128, 1)).astype(np.float32))
        in_maps.append(m)

    res = run_bass_kernel_spmd(nc, in_maps, core_ids=list(range(NCORES))).results

    tr = lambda k: np.ascontiguousarray(
        np.concatenate([r[k] for r in res], 0).transpose(0, 2, 1))
    o = tr('o_td')
    z_p = tr('zp_td')
    z_c = tr('zc_td')
    z_r = tr('zr_td')
    timbre = np.concatenate([r['timbre_o'] for r in res], 0)
    sse = np.float32(sum(float(r['sse_o'].sum(dtype=np.float64)) for r in res))
    commit = np.float32(sse / np.float32(B * CB_D * T))
    return (o, z_p, z_c, z_r, commit, commit, timbre)


# revision 13
# speedup vs baseline: 22.3582x; 22.3582x over previous
"""TRN2 Bass kernel for nn_FAquantizer (vq_codebook).

Data-parallel over batch B=16 across 8 NeuronCores (2 batches/core).

Numerics strategy (validated against the fp32 jax reference):
- WaveNet convs / residual-skip / projections / style encoder: 3-pass
  split-FP22 matmuls (float32r, round-to-nearest-even @ 11 mantissa bits)
  -> fp32-grade accuracy at 1 cycle/row per pass.
- VQ score path (z_e bases, cosine scores): true fp32 matmuls (bit-exact
  operands).
- RVQ residual chain: algebraic rewrite. z_e for later quantizers =
  base matmul vs the branch input + small 8-dim gathered corrections
  from host-precomputed tables; z_q outputs are row gathers from
  host-precomputed (codebook @ out_w.T + out_b) tables.
- Losses: commitment == codebook loss (forward); single SSE accumulator.
- Final AdaIN-LN computed in (t, d) layout; host transposes outputs.
"""
import numpy as np
from contextlib import ExitStack

import concourse.bass as bass
import concourse.bacc as bacc
import concourse.mybir as mybir
import concourse.tile as tile
from concourse.bass import ts
from concourse.bass_utils import run_bass_kernel_spmd

F32 = mybir.dt.float32
F32R = mybir.dt.float32r
U32 = mybir.dt.uint32
AF = mybir.ActivationFunctionType
ALU = mybir.AluOpType

B, T, D, H = 16, 1024, 1024, 256
CB_K, CB_D = 1024, 8
NCORES = 8
BL = B // NCORES          # batches per core = 2
NT = T // 128             # 8 t-tiles
ND = D // 128             # 8 d-tiles
NL_P, NL_C = 8, 16        # wavenet layers

# quantizer graph: global order p0, c0, c1, r0, r1, r2
SUCC = {0: [3, 4, 5], 1: [2, 3, 4, 5], 2: [3, 4, 5], 3: [4, 5], 4: [5], 5: []}
PRED = {0: [], 1: [], 2: [1], 3: [0, 1, 2], 4: [0, 1, 2, 3], 5: [0, 1, 2, 3, 4]}
SMALL_W = {q: 8 * len(SUCC[q]) for q in range(5)}

# tunables
WNW_BUFS = 2
XSP_BUFS = 9


def r22(x):
    """float32r rounding: round-to-nearest-even at 11 mantissa bits."""
    x = np.asarray(x, np.float32)
    xa = np.abs(x).view(np.int32).astype(np.int64)
    step = 1 << 12
    q, rem = np.divmod(xa, step)
    up = (rem > step // 2) | ((rem == step // 2) & ((q & 1) == 1))
    r = ((q + up) * step).astype(np.int64)
    out = r.astype(np.int32).view(np.float32).copy()
    return (out * np.where(np.signbit(x), np.float32(-1), np.float32(1))).astype(np.float32)


def split22(x):
    hi = r22(x)
    lo = r22((np.asarray(x, np.float32) - hi).astype(np.float32))
    return hi, lo


def _blocks_flat(wT, nk=None):
    """wT: (K, M) lhsT -> hi/lo split, tiled into (P, nblk*128) flat layout.

    Returns array (P, 2 * nKblk * nMblk * 128) with block order (s, kb, mb),
    P = min(K, 128) partition rows per block... K must be multiple of 128 or
    <=128. Here handled by caller; this helper is for K multiple of 128.
    """
    raise NotImplementedError


def prep_conv_blocks(w_list, P):
    """w_list: list over block-cols of (P, 128) fp32 arrays (already split).
    -> (P, len*128) flat."""
    return np.concatenate(w_list, axis=1).astype(np.float32)


def host_prepare(inputs):
    """Precompute all device-side constant tensors (shared across cores)."""
    f32 = lambda k: np.ascontiguousarray(np.asarray(inputs[k], np.float32))
    prep = {}

    def conv_w_layout(in_w, in_b):
        """in_w (L, 2H, H, 5) -> (L, 2, 128, 40*128) f32r-split block layout,
        in_b -> (L, 128, 4)."""
        L = in_w.shape[0]
        out = np.zeros((L, 2, 128, 40 * 128), np.float32)
        for l in range(L):
            for k in range(5):
                wT = in_w[l, :, :, k].T.astype(np.float32)   # (256, 512)
                hi, lo = split22(wT)
                for mh in range(2):
                    for s, w_s in enumerate((hi, lo)):
                        for c in range(2):
                            for m2 in range(2):
                                m = mh * 2 + m2
                                blk = ((s * 5 + k) * 2 + c) * 2 + m2
                                out[l, mh, :, blk * 128:(blk + 1) * 128] = \
                                    w_s[c * 128:(c + 1) * 128, m * 128:(m + 1) * 128]
        bias = np.zeros((L, 128, 4), np.float32)
        for l in range(L):
            bias[l] = in_b[l].reshape(4, 128).T
        return out, bias

    def rs_w_layout(rs_w, rs_b):
        """rs_w (L, 2H, H) -> (L, 128, 16*128); rs_b -> (L, 128, 4)."""
        L = rs_w.shape[0]
        out = np.zeros((L, 128, 16 * 128), np.float32)
        for l in range(L):
            hi, lo = split22(rs_w[l].T.astype(np.float32))  # (256, 512)
            for s, w_s in enumerate((hi, lo)):
                for c in range(2):
                    for m in range(4):
                        blk = (s * 2 + c) * 4 + m
                        out[l, :, blk * 128:(blk + 1) * 128] = \
                            w_s[c * 128:(c + 1) * 128, m * 128:(m + 1) * 128]
        bias = np.zeros((L, 128, 4), np.float32)
        for l in range(L):
            bias[l] = rs_b[l].reshape(4, 128).T
        return out, bias

    def proj_layout(w):
        """w (M, K) -> lhsT (K, M) split into (K<=128 rows, 2*nc*nm*128)."""
        wT = w.T.astype(np.float32)  # (K, M)
        K, M = wT.shape
        hi, lo = split22(wT)
        ncb = max(1, K // 128)
        nmb = M // 128
        P = min(K, 128)
        blks = []
        for s, w_s in enumerate((hi, lo)):
            for c in range(ncb):
                for m in range(nmb):
                    blks.append(w_s[c * 128:c * 128 + P, m * 128:(m + 1) * 128])
        return np.concatenate(blks, axis=1).astype(np.float32)

    def bias_cols(b):
        """(M,) -> (128, M/128)"""
        return np.ascontiguousarray(b.reshape(-1, 128).T.astype(np.float32))

    # wavenet weights (prosody 8 + content 16 concatenated)
    mw, mb = conv_w_layout(f32('mwn_in_w'), f32('mwn_in_b'))
    cw, cb_ = conv_w_layout(f32('cwn_in_w'), f32('cwn_in_b'))
    prep['wn_w'] = np.concatenate([mw, cw], 0)
    prep['wn_b'] = np.concatenate([mb, cb_], 0)
    mrw, mrb = rs_w_layout(f32('mwn_rs_w'), f32('mwn_rs_b'))
    crw, crb = rs_w_layout(f32('cwn_rs_w'), f32('cwn_rs_b'))
    prep['rs_w'] = np.concatenate([mrw, crw], 0)
    prep['rs_b'] = np.concatenate([mrb, crb], 0)

    prep['ml_w'] = proj_layout(f32('ml_w'))          # (20, 2*1*2*128=512)
    prep['ml_b'] = bias_cols(f32('ml_b'))            # (128, 2)
    W_mc = (np.float64(f32('cl_w')) @ np.float64(f32('dct_mat')).T).astype(np.float32)
    prep['W_mc'] = proj_layout(W_mc)                 # (80, 512)
    prep['cl_b'] = bias_cols(f32('cl_b'))
    prep['ml2_w'] = proj_layout(f32('ml2_w'))        # (128, 2*2*8*128=4096)
    prep['ml2_b'] = bias_cols(f32('ml2_b'))          # (128, 8)
    prep['cl2_w'] = proj_layout(f32('cl2_w'))
    prep['cl2_b'] = bias_cols(f32('cl2_b'))

    # style encoder
    se1 = np.zeros((80, 20 * 128), np.float32)
    for k in range(5):
        hi, lo = split22(f32('se_w1')[:, :, k].T)    # (80, 256)
        for s, w_s in enumerate((hi, lo)):
            for m in range(2):
                blk = (s * 5 + k) * 2 + m
                se1[:, blk * 128:(blk + 1) * 128] = w_s[:, m * 128:(m + 1) * 128]
    prep['se_w1'] = se1
    prep['se_b1'] = bias_cols(f32('se_b1'))
    se2 = np.zeros((128, 40 * 128), np.float32)
    for k in range(5):
        hi, lo = split22(f32('se_w2')[:, :, k].T)    # (256, 256)
        for s, w_s in enumerate((hi, lo)):
            for c in range(2):
                for m in range(2):
                    blk = ((s * 5 + k) * 2 + c) * 2 + m
                    se2[:, blk * 128:(blk + 1) * 128] = \
                        w_s[c * 128:(c + 1) * 128, m * 128:(m + 1) * 128]
    prep['se_w2'] = se2
    prep['se_b2'] = bias_cols(f32('se_b2'))
    prep['se_wo'] = proj_layout(f32('se_wo'))        # (128, 2*2*8*128)
    prep['se_bo'] = bias_cols(f32('se_bo'))
    Wst = (np.float64(f32('tl_w')) @ np.float64(f32('se_wo'))).astype(np.float32)
    bst = (np.float64(f32('tl_b')) + np.float64(f32('tl_w')) @ np.float64(f32('se_bo'))).astype(np.float32)
    prep['Wst'] = proj_layout(Wst)                   # (128, 2*2*16*128=8192)
    prep['bst'] = bias_cols(bst)                     # (128, 16)

    # RVQ tables
    groups = [('p', 0), ('c', 0), ('c', 1), ('r', 0), ('r', 1), ('r', 2)]
    in_ws, in_bs, cbs, out_rows64 = [], [], [], []
    for g, i in groups:
        in_ws.append(f32(f'{g}_in_w')[i])
        in_bs.append(f32(f'{g}_in_b')[i])
        cbs.append(f32(f'{g}_cb')[i])
        o64 = np.float64(cbs[-1]) @ np.float64(f32(f'{g}_out_w')[i]).T \
            + np.float64(f32(f'{g}_out_b')[i])[None, :]
        out_rows64.append(o64)

    zin = np.zeros((128, 6 * 8 * 8), np.float32)
    for q in range(6):
        wT = in_ws[q].T.astype(np.float32)           # (1024, 8)
        for dt in range(ND):
            zin[:, (q * 8 + dt) * 8:(q * 8 + dt + 1) * 8] = wT[dt * 128:(dt + 1) * 128]
    prep['zin_w'] = zin
    zb = np.zeros((128, 48), np.float32)
    for q in range(6):
        zb[:, q * 8:(q + 1) * 8] = in_bs[q][None, :]
    prep['zin_b'] = zb

    cbn = np.zeros((6, 8, 1024), np.float32)
    for q in range(6):
        n = np.maximum(np.linalg.norm(cbs[q], axis=-1, keepdims=True),
                       np.float32(1e-12)).astype(np.float32)
        cbn[q] = (cbs[q] / n).T
    prep['cbn'] = cbn

    for q in range(6):
        big = np.zeros((CB_K, 1032), np.float32)
        big[:, :1024] = out_rows64[q].astype(np.float32)
        big[:, 1024:] = cbs[q]
        prep[f'big{q}'] = big
    for q in range(5):
        sm = np.zeros((CB_K, SMALL_W[q]), np.float32)
        for j, s in enumerate(SUCC[q]):
            sm[:, 8 * j:8 * (j + 1)] = (out_rows64[q] @ np.float64(in_ws[s]).T).astype(np.float32)
        prep[f'small{q}'] = sm

    prep['ident'] = np.eye(128, dtype=np.float32)
    return prep


def build_nc():
    nc = bacc.Bacc("TRN2", debug=False, num_devices=NCORES)
    P = {}

    def par(name, shape, dtype=F32, out=False):
        P[name] = nc.declare_dram_parameter(name, list(shape), dtype, isOutput=out)
        return P[name]

    # inputs (per-core slices)
    par('x', (BL, D, T))
    par('prosody', (BL, 20, T))
    par('mel', (BL, 80, T))
    par('maskp', (BL, 128, 1))
    # weights
    par('wn_w', (24, 2, 128, 40 * 128), F32R)
    par('wn_b', (24, 128, 4))
    par('rs_w', (24, 128, 16 * 128), F32R)
    par('rs_b', (24, 128, 4))
    par('ml_w', (20, 512), F32R); par('ml_b', (128, 2))
    par('W_mc', (80, 512), F32R); par('cl_b', (128, 2))
    par('ml2_w', (128, 4096), F32R); par('ml2_b', (128, 8))
    par('cl2_w', (128, 4096), F32R); par('cl2_b', (128, 8))
    par('se_w1', (80, 2560), F32R); par('se_b1', (128, 2))
    par('se_w2', (128, 5120), F32R); par('se_b2', (128, 2))
    par('se_wo', (128, 4096), F32R); par('se_bo', (128, 8))
    par('Wst', (128, 8192), F32R); par('bst', (128, 16))
    par('zin_w', (128, 384)); par('zin_b', (128, 48))
    par('cbn', (6, 8, 1024))
    for q in range(6):
        par(f'big{q}', (CB_K, 1032))
    for q in range(5):
        par(f'small{q}', (CB_K, SMALL_W[q]))
    par('ident', (128, 128))
    # outputs
    par('o_td', (BL, T, D), out=True)
    par('zp_td', (BL, T, D), out=True)
    par('zc_td', (BL, T, D), out=True)
    par('zr_td', (BL, T, D), out=True)
    par('timbre_o', (BL, 1024), out=True)
    par('sse_o', (128, 1), out=True)

    ap = lambda n: P[n].ap()

    with tile.TileContext(nc) as tc, ExitStack() as ctx:
        # ---- persistent pools ----
        const = ctx.enter_context(tc.tile_pool(name="const", bufs=1))
        psA = ctx.enter_context(tc.tile_pool(name="psA", bufs=2, space="PSUM"))   # conv(3), rs(2)
        psB = ctx.enter_context(tc.tile_pool(name="psB", bufs=2, space="PSUM"))   # zb
        psC = ctx.enter_context(tc.tile_pool(name="psC", bufs=1, space="PSUM"))   # tp
        psD = ctx.enter_context(tc.tile_pool(name="psD", bufs=1, space="PSUM"))   # sc
        dram = ctx.enter_context(tc.tile_pool(name="dram", bufs=1, space="DRAM"))
        persist = ctx.enter_context(tc.tile_pool(name="persist", bufs=1))

        identt = const.tile([128, 128], F32)
        nc.sync.dma_start(identt[:], ap('ident'))
        zinw = const.tile([128, 384], F32)
        nc.sync.dma_start(zinw[:], ap('zin_w'))
        zinb = const.tile([128, 48], F32)
        nc.sync.dma_start(zinb[:], ap('zin_b'))
        maskt = const.tile([128, BL], F32)
        for b in range(BL):
            nc.sync.dma_start(maskt[:, b:b + 1], ap('maskp')[b])

        zeT = persist.tile([128, 6 * BL * NT * 8], F32, name="zeT")       # 3KB/p
        idxT = persist.tile([128, 6 * BL * NT], U32, name="idxT")
        sseT = persist.tile([128, 6 * BL * NT], F32, name="sseT")
        smal = persist.tile([128, 5 * BL * NT * 32], F32, name="smal")     # corr gathers

        chn = ctx.enter_context(tc.tile_pool(name="chn", bufs=2))
        cbn_pool = ctx.enter_context(tc.tile_pool(name="cbnp", bufs=1))

        zcol = lambda q, b, t: ((q * BL + b) * NT + t) * 8
        icol = lambda q, b, t: (q * BL + b) * NT + t
        scol = lambda q, b, t: ((q * BL + b) * NT + t) * 32

        # style row scratch in DRAM
        st_scr = [dram.tile([1, 2048], F32, name=f"stscr{b}", tag=f"stscr{b}") for b in range(BL)]
        # f0/cont scratch
        br_scr = {}
        for b in range(BL):
            for m in range(ND):
                br_scr[(b, m)] = dram.tile([128, T], F32, name=f"brscr{b}_{m}", tag=f"brscr{b}_{m}")

        # ================= helpers =================
        def mm3_list(psum, triples, n=512):
            """triples: list of (whi, wlo, xr, xlo) matmuls accumulated; each
            expands to 3 passes."""
            mms = []
            for whi, wlo, xr, xlo in triples:
                mms.append((whi, xr))
                mms.append((whi, xlo))
                mms.append((wlo, xr))
            for i, (lh, rh) in enumerate(mms):
                nc.tensor.matmul(psum, lh, rh, start=(i == 0), stop=(i == len(mms) - 1))

        def split_dev(pool, src_ap, shape, tag):
            """returns (hi_f32r_tile, lo_f32r_tile) of src_ap."""
            hi = pool.tile(shape, F32R, name=f"hi_{tag}", tag="xsp")
            lo = pool.tile(shape, F32R, name=f"lo_{tag}", tag="xsp")
            nc.vector.tensor_copy(hi[:], src_ap)
            nc.vector.tensor_sub(lo[:], src_ap, hi[:].bitcast(F32))
            return hi, lo

        # ================= P1: style encoder =================
        with tc.tile_pool(name="p1", bufs=1) as p1, \
             tc.tile_pool(name="melp", bufs=1) as melp:
            se1t = p1.tile([80, 2560], F32R)
            nc.sync.dma_start(se1t[:], ap('se_w1'))
            se2t = p1.tile([128, 5120], F32R)
            nc.sync.dma_start(se2t[:], ap('se_w2'))
            sewot = p1.tile([128, 4096], F32R)
            nc.sync.dma_start(sewot[:], ap('se_wo'))
            wstt = p1.tile([128, 8192], F32R)
            nc.sync.dma_start(wstt[:], ap('Wst'))
            seb1 = p1.tile([128, 2], F32); nc.sync.dma_start(seb1[:], ap('se_b1'))
            seb2 = p1.tile([128, 2], F32); nc.sync.dma_start(seb2[:], ap('se_b2'))
            sebo = p1.tile([128, 8], F32); nc.sync.dma_start(sebo[:], ap('se_bo'))
            bstt = p1.tile([128, 16], F32); nc.sync.dma_start(bstt[:], ap('bst'))

            melt, melr, mello = {}, {}, {}
            for b in range(BL):
                mt = melp.tile([80, T + 4], F32, name=f"mel{b}", tag=f"mel{b}")
                nc.gpsimd.memset(mt[:, 0:2], 0.0)
                nc.gpsimd.memset(mt[:, T + 2:T + 4], 0.0)
                nc.sync.dma_start(mt[:, 2:T + 2], ap('mel')[b])
                mr = melp.tile([80, T + 4], F32R, name=f"melr{b}", tag=f"melr{b}")
                ml = melp.tile([80, T + 4], F32R, name=f"mello{b}", tag=f"mello{b}")
                nc.vector.tensor_copy(mr[:], mt[:])
                nc.vector.tensor_sub(ml[:], mt[:], mr[:].bitcast(F32))
                melt[b], melr[b], mello[b] = mt, mr, ml

            h1, h1r, h1lo = {}, {}, {}
            h2 = {}
            for b in range(BL):
                for m in range(2):
                    t_ = p1.tile([128, T + 4], F32, name=f"h1_{b}_{m}", tag=f"h1_{b}_{m}")
                    nc.gpsimd.memset(t_[:, 0:2], 0.0)
                    nc.gpsimd.memset(t_[:, T + 2:T + 4], 0.0)
                    h1[(b, m)] = t_
            # conv1 (contract 80)
            blk1 = lambda s, k, m: (s * 5 + k) * 2 + m
            for b in range(BL):
                for m in range(2):
                    for n in range(2):
                        ps = psA.tile([128, 512], F32, tag="conv")
                        mms = []
                        for s_w, x_t in ((0, melr[b]), (0, mello[b]), (1, melr[b])):
                            for k in range(5):
                                mms.append((se1t[:, ts(blk1(s_w, k, m), 128)],
                                            x_t[:, n * 512 + k: n * 512 + k + 512]))
                        for i, (lh, rh) in enumerate(mms):
                            nc.tensor.matmul(ps[:], lh, rh, start=(i == 0), stop=(i == len(mms) - 1))
                        # bias + leaky relu 0.2 into padded h1
                        dst = h1[(b, m)][:, 2 + n * 512: 2 + n * 512 + 512]
                        nc.scalar.activation(dst, ps[:], AF.Identity, bias=seb1[:, m:m + 1])
                        nc.vector.scalar_tensor_tensor(
                            out=dst, in0=dst, scalar=0.2, in1=dst,
                            op0=ALU.mult, op1=ALU.max)
            for b in range(BL):
                for m in range(2):
                    r_ = p1.tile([128, T + 4], F32R, name=f"h1r_{b}_{m}", tag=f"h1r_{b}_{m}")
                    l_ = p1.tile([128, T + 4], F32R, name=f"h1l_{b}_{m}", tag=f"h1l_{b}_{m}")
                    nc.vector.tensor_copy(r_[:], h1[(b, m)][:])
                    nc.vector.tensor_sub(l_[:], h1[(b, m)][:], r_[:].bitcast(F32))
                    h1r[(b, m)], h1lo[(b, m)] = r_, l_
            # conv2 (contract 256)
            blk2 = lambda s, k, c, m: ((s * 5 + k) * 2 + c) * 2 + m
            for b in range(BL):
                for m in range(2):
                    h2[(b, m)] = p1.tile([128, T], F32, name=f"h2_{b}_{m}", tag=f"h2_{b}_{m}")
                    for n in range(2):
                        ps = psA.tile([128, 512], F32, tag="conv")
                        mms = []
                        for s_w, hr, hl in ((0, h1r, None), (0, None, h1lo), (1, h1r, None)):
                            for k in range(5):
                                for c in range(2):
                                    x_t = hr[(b, c)] if hr is not None else hl[(b, c)]
                                    mms.append((se2t[:, ts(blk2(s_w, k, c, m), 128)],
                                                x_t[:, n * 512 + k: n * 512 + k + 512]))
                        for i, (lh, rh) in enumerate(mms):
                            nc.tensor.matmul(ps[:], lh, rh, start=(i == 0), stop=(i == len(mms) - 1))
                        dst = h2[(b, m)][:, ts(n, 512)]
                        nc.scalar.activation(dst, ps[:], AF.Identity, bias=seb2[:, m:m + 1])
                        nc.vector.scalar_tensor_tensor(
                            out=dst, in0=dst, scalar=0.2, in1=dst,
                            op0=ALU.mult, op1=ALU.max)
            # mean over T, then timbre / style matvecs (N=2: both batches)
            hmall = p1.tile([128, 4], F32, name="hmall", tag="hmall")  # col = c*2+b
            for b in range(BL):
                for m in range(2):
                    s_ = p1.tile([128, 1], F32, name=f"hs{b}{m}", tag="hs")
                    nc.vector.reduce_sum(s_[:], h2[(b, m)][:], axis=mybir.AxisListType.X)
                    nc.scalar.mul(hmall[:, m * 2 + b:m * 2 + b + 1], s_[:], 1.0 / T)
            hmr = p1.tile([128, 4], F32R, name="hmr", tag="hmr")
            hmlo = p1.tile([128, 4], F32R, name="hmlo", tag="hmlo")
            nc.vector.tensor_copy(hmr[:], hmall[:])
            nc.vector.tensor_sub(hmlo[:], hmall[:], hmr[:].bitcast(F32))
            pblk = lambda s, c, m, nm: (s * 2 + c) * nm + m
            timball = p1.tile([128, 16], F32, name="timball", tag="timball")  # col m*2+b
            for m in range(8):
                ps = psB.tile([128, 8], F32, tag="zb")
                mms = []
                for s_w, x_t in ((0, hmr), (0, hmlo), (1, hmr)):
                    for c in range(2):
                        mms.append((sewot[:, ts(pblk(s_w, c, m, 8), 128)],
                                    x_t[:, c * 2:c * 2 + 2]))
                for i, (lh, rh) in enumerate(mms):
                    nc.tensor.matmul(ps[:, 0:2], lh, rh, start=(i == 0), stop=(i == len(mms) - 1))
                nc.scalar.activation(timball[:, m * 2:m * 2 + 2], ps[:, 0:2], AF.Identity,
                                     bias=sebo[:, m:m + 1])
            stall = p1.tile([128, 32], F32, name="stall", tag="stall")  # col m*2+b
            for m in range(16):
                ps = psB.tile([128, 8], F32, tag="zb")
                mms = []
                for s_w, x_t in ((0, hmr), (0, hmlo), (1, hmr)):
                    for c in range(2):
                        mms.append((wstt[:, ts(pblk(s_w, c, m, 16), 128)],
                                    x_t[:, c * 2:c * 2 + 2]))
                for i, (lh, rh) in enumerate(mms):
                    nc.tensor.matmul(ps[:, 0:2], lh, rh, start=(i == 0), stop=(i == len(mms) - 1))
                nc.scalar.activation(stall[:, m * 2:m * 2 + 2], ps[:, 0:2], AF.Identity,
                                     bias=bstt[:, m:m + 1])
            for b in range(BL):
                nc.sync.dma_start(
                    ap('timbre_o')[b:b + 1, :].rearrange("o (m p) -> (o p) m", p=128),
                    timball[:, :].rearrange("p (m b) -> p b m", b=2)[:, b])
                nc.sync.dma_start(
                    st_scr[b][0:1, :].rearrange("o (m p) -> (o p) m", p=128),
                    stall[:, :].rearrange("p (m b) -> p b m", b=2)[:, b])

        # ================= WaveNet machinery =================
        wn_stack = ExitStack()
        wn = wn_stack.enter_context(tc.tile_pool(name="wn", bufs=1))
        wwp = wn_stack.enter_context(tc.tile_pool(name="wwp", bufs=WNW_BUFS))
        xsp = wn_stack.enter_context(tc.tile_pool(name="xsp", bufs=XSP_BUFS))

        xw = {}
        ow = {}
        for b in range(BL):
            for c2 in range(2):
                t_ = wn.tile([128, T + 4], F32, name=f"xw{b}{c2}", tag=f"xw{b}{c2}")
                nc.gpsimd.memset(t_[:, 0:2], 0.0)
                nc.gpsimd.memset(t_[:, T + 2:T + 4], 0.0)
                xw[(b, c2)] = t_
                ow[(b, c2)] = wn.tile([128, T], F32, name=f"ow{b}{c2}", tag=f"ow{b}{c2}")

        cblk = lambda s, k, c, m2: ((s * 5 + k) * 2 + c) * 2 + m2
        rblk = lambda s, c, m: (s * 2 + c) * 4 + m

        def wavenet_layer(l, last):
            wc = [wwp.tile([128, 40 * 128], F32R, name=f"wc{l}_{mh}", tag="wnw", bufs=3)
                  for mh in range(2)]
            nc.sync.dma_start(wc[0][:], ap('wn_w')[l, 0])
            nc.sync.dma_start(wc[1][:], ap('wn_w')[l, 1])
            rc = wwp.tile([128, 16 * 128], F32R, name=f"rc{l}", tag="rsw", bufs=1)
            nc.sync.dma_start(rc[:], ap('rs_w')[l])
            wb = wwp.tile([128, 4], F32, name=f"wb{l}", tag="wnb")
            nc.sync.dma_start(wb[:], ap('wn_b')[l])
            rb = wwp.tile([128, 4], F32, name=f"rb{l}", tag="rsb")
            nc.sync.dma_start(rb[:], ap('rs_b')[l])

            xr, xlo = {}, {}
            for b in range(BL):
                for c in range(2):
                    xr[(b, c)], xlo[(b, c)] = split_dev(xsp, xw[(b, c)][:], [128, T + 4], f"x{l}{b}{c}")
            sg = {}
            acts = {}
            for b in range(BL):
                for c in range(2):
                    sg[(b, c)] = xsp.tile([128, T], F32, name=f"sg{l}{b}{c}", tag="xsp")
                    acts[(b, c)] = xsp.tile([128, T], F32, name=f"ac{l}{b}{c}", tag="acts", bufs=4)
            for mh in range(2):
                for b in range(BL):
                    for m2 in range(2):
                        m = mh * 2 + m2
                        for n in range(2):
                            ps = psA.tile([128, 512], F32, tag="conv")
                            mms = []
                            for s_w, xd in ((0, xr), (0, xlo), (1, xr)):
                                for k in range(5):
                                    for c in range(2):
                                        mms.append((wc[mh][:, ts(cblk(s_w, k, c, m2), 128)],
                                                    xd[(b, c)][:, n * 512 + k:n * 512 + k + 512]))
                            for i, (lh, rh) in enumerate(mms):
                                nc.tensor.matmul(ps[:], lh, rh, start=(i == 0), stop=(i == len(mms) - 1))
                            if m < 2:
                                nc.scalar.activation(acts[(b, m)][:, ts(n, 512)], ps[:],
                                                     AF.Tanh, bias=wb[:, m:m + 1])
                            else:
                                nc.scalar.activation(sg[(b, m - 2)][:, ts(n, 512)], ps[:],
                                                     AF.Sigmoid, bias=wb[:, m:m + 1])
            ar, alo = {}, {}
            for b in range(BL):
                for c in range(2):
                    nc.vector.tensor_mul(acts[(b, c)][:], acts[(b, c)][:], sg[(b, c)][:])
                    ar[(b, c)], alo[(b, c)] = split_dev(xsp, acts[(b, c)][:], [128, T], f"a{l}{b}{c}")
            for b in range(BL):
                for m in range(4):
                    for n in range(2):
                        if m < 2 and last:
                            continue
                        ps = psA.tile([128, 512], F32, tag="rs")
                        mms = []
                        for s_w, ad in ((0, ar), (0, alo), (1, ar)):
                            for c in range(2):
                                mms.append((rc[:, ts(rblk(s_w, c, m), 128)],
                                            ad[(b, c)][:, ts(n, 512)]))
                        for i, (lh, rh) in enumerate(mms):
                            nc.tensor.matmul(ps[:], lh, rh, start=(i == 0), stop=(i == len(mms) - 1))
                        if m < 2:
                            dst = xw[(b, m)][:, 2 + n * 512: 2 + n * 512 + 512]
                        else:
                            dst = ow[(b, m - 2)][:, ts(n, 512)]
                        nc.vector.scalar_tensor_tensor(
                            out=dst, in0=ps[:], scalar=rb[:, m:m + 1], in1=dst,
                            op0=ALU.add, op1=ALU.add)

        def branch_proj(wname, bname, dst_scr):
            """project ow (2H=... here h=256 -> 1024) via 1024x256 matrix."""
            wt = wn.tile([128, 4096], F32R, name=f"proj_{wname}", tag="projw")
            nc.sync.dma_start(wt[:], ap(wname))
            bt = wn.tile([128, 8], F32, name=f"projb_{bname}", tag="projb")
            nc.sync.dma_start(bt[:], ap(bname))
            orr, olo = {}, {}
            for b in range(BL):
                for c in range(2):
                    orr[(b, c)], olo[(b, c)] = split_dev(xsp, ow[(b, c)][:], [128, T], f"o{wname}{b}{c}")
            pb = lambda s, c, m: (s * 2 + c) * 8 + m
            for b in range(BL):
                for m in range(8):
                    ft = xsp.tile([128, T], F32, name=f"ft{wname}{b}{m}", tag="ftmp", bufs=2)
                    for n in range(2):
                        ps = psA.tile([128, 512], F32, tag="conv")
                        mms = []
                        for s_w, od in ((0, orr), (0, olo), (1, orr)):
                            for c in range(2):
                                mms.append((wt[:, ts(pb(s_w, c, m), 128)],
                                            od[(b, c)][:, ts(n, 512)]))
                        for i, (lh, rh) in enumerate(mms):
                            nc.tensor.matmul(ps[:], lh, rh, start=(i == 0), stop=(i == len(mms) - 1))
                        nc.scalar.activation(ft[:, ts(n, 512)], ps[:], AF.Identity,
                                             bias=bt[:, m:m + 1])
                    nc.sync.dma_start(dst_scr[(b, m)][:], ft[:])

        # ---- chain helpers ----
        def zbase(qlist, src_of_bt):
            """compute z_eT bases for quantizers qlist; src_of_bt(b, dt, tt) -> lhsT AP."""
            for b in range(BL):
                for tt in range(NT):
                    blks = []
                    for dt in range(ND):
                        bl_ = chn.tile([128, 128], F32, name=f"zb{qlist[0]}{b}{tt}{dt}", tag="zblk", bufs=10)
                        nc.sync.dma_start(bl_[:], src_of_bt(b, dt, tt))
                        blks.append(bl_)
                    for q in qlist:
                        ps = psB.tile([128, 8], F32, tag="zb")
                        for dt in range(ND):
                            nc.tensor.matmul(ps[:], blks[dt][:],
                                             zinw[:, ts(q * 8 + dt, 8)],
                                             start=(dt == 0), stop=(dt == ND - 1))
                        nc.vector.tensor_add(zeT[:, ts(zcol(q, b, tt) // 8, 8)],
                                             ps[:], zinb[:, ts(q, 8)])

        def chain_q(q, cbnt):
            """corrections, scores, argmax, small-gather for quantizer q."""
            for b in range(BL):
                for tt in range(NT):
                    zslice = zeT[:, ts(zcol(q, b, tt) // 8, 8)]
                    for pq in PRED[q]:
                        off = 8 * SUCC[pq].index(q)
                        nc.vector.tensor_sub(
                            zslice, zslice,
                            smal[:, scol(pq, b, tt) + off: scol(pq, b, tt) + off + 8])
                    # transpose to (8, t)
                    pt = psC.tile([8, 128], F32, tag="tp")
                    nc.tensor.transpose(pt[:], zslice, identt[:])
                    zet = chn.tile([8, 128], F32, name=f"zet{q}{b}{tt}", tag="zes", bufs=2)
                    nc.vector.tensor_copy(zet[:], pt[:])
                    sc = chn.tile([128, 1024], F32, name=f"sc{q}{b}{tt}", tag="scsb", bufs=1)
                    for hf in range(2):
                        psc = psD.tile([128, 512], F32, tag="sc")
                        nc.tensor.matmul(psc[:], zet[:], cbnt[:, ts(hf, 512)],
                                         start=True, stop=True)
                        nc.vector.tensor_copy(sc[:, ts(hf, 512)], psc[:])
                    mx = chn.tile([128, 8], F32, name=f"mx{q}{b}{tt}", tag="mx", bufs=2)
                    ix = chn.tile([128, 8], U32, name=f"ix{q}{b}{tt}", tag="ix", bufs=2)
                    nc.vector.max(mx[:], sc[:])
                    nc.vector.max_index(ix[:], mx[:], sc[:])
                    nc.vector.tensor_copy(idxT[:, icol(q, b, tt):icol(q, b, tt) + 1],
                                          ix[:, 0:1])
                    if q < 5:
                        w = SMALL_W[q]
                        nc.gpsimd.indirect_dma_start(
                            out=smal[:, scol(q, b, tt):scol(q, b, tt) + w],
                            out_offset=None,
                            in_=ap(f'small{q}'),
                            in_offset=bass.IndirectOffsetOnAxis(
                                ap=idxT[:, icol(q, b, tt):icol(q, b, tt) + 1], axis=0))

        def load_cbn(q):
            t_ = cbn_pool.tile([8, 1024], F32, name=f"cbn{q}", tag="cbn")
            nc.sync.dma_start(t_[:], ap('cbn')[q])
            return t_

        # ================= P2: prosody =================
        if True:
            mlwt = xsp.tile([20, 512], F32R, name="mlwt", tag="xsp")
            nc.sync.dma_start(mlwt[:], ap('ml_w'))
            mlbt = chn.tile([128, 2], F32, name="mlbt", tag="mx", bufs=2)
            nc.sync.dma_start(mlbt[:], ap('ml_b'))
            prt = {}
            for b in range(BL):
                pt_ = xsp.tile([20, T], F32, name=f"pr{b}", tag="xsp")
                nc.sync.dma_start(pt_[:], ap('prosody')[b])
                prr = xsp.tile([20, T], F32R, name=f"prr{b}", tag="xsp")
                prl = xsp.tile([20, T], F32R, name=f"prl{b}", tag="xsp")
                nc.vector.tensor_copy(prr[:], pt_[:])
                nc.vector.tensor_sub(prl[:], pt_[:], prr[:].bitcast(F32))
                prt[b] = (prr, prl)
            for b in range(BL):
                for m in range(2):
                    for n in range(2):
                        ps = psA.tile([128, 512], F32, tag="conv")
                        mms = []
                        for s_w, xi in ((0, 0), (0, 1), (1, 0)):
                            mms.append((mlwt[:, ts(s_w * 2 + m, 128)],
                                        prt[b][xi][:, ts(n, 512)]))
                        for i, (lh, rh) in enumerate(mms):
                            nc.tensor.matmul(ps[:], lh, rh, start=(i == 0), stop=(i == len(mms) - 1))
                        nc.scalar.activation(xw[(b, m)][:, 2 + n * 512:2 + n * 512 + 512],
                                             ps[:], AF.Identity, bias=mlbt[:, m:m + 1])
            for b in range(BL):
                for c in range(2):
                    nc.gpsimd.memset(ow[(b, c)][:], 0.0)
        for l in range(NL_P):
            wavenet_layer(l, last=(l == NL_P - 1))
        branch_proj('ml2_w', 'ml2_b', br_scr)
        zbase([0], lambda b, dt, tt: br_scr[(b, dt)][:, ts(tt, 128)])
        chain_q(0, load_cbn(0))

        # ================= P3: content =================
        if True:
            wmct = xsp.tile([80, 512], F32R, name="wmct", tag="xsp")
            nc.sync.dma_start(wmct[:], ap('W_mc'))
            clbt = chn.tile([128, 2], F32, name="clbt", tag="mx", bufs=2)
            nc.sync.dma_start(clbt[:], ap('cl_b'))
            m3r, m3l = {}, {}
            for b in range(BL):
                m3 = xsp.tile([80, T], F32, name=f"m3{b}", tag="xsp")
                nc.sync.dma_start(m3[:], ap('mel')[b])
                m3r[b] = xsp.tile([80, T], F32R, name=f"m3r{b}", tag="xsp")
                m3l[b] = xsp.tile([80, T], F32R, name=f"m3l{b}", tag="xsp")
                nc.vector.tensor_copy(m3r[b][:], m3[:])
                nc.vector.tensor_sub(m3l[b][:], m3[:], m3r[b][:].bitcast(F32))
            for b in range(BL):
                for m in range(2):
                    for n in range(2):
                        ps = psA.tile([128, 512], F32, tag="conv")
                        mms = []
                        for s_w, x_t in ((0, m3r[b]), (0, m3l[b]), (1, m3r[b])):
                            mms.append((wmct[:, ts(s_w * 2 + m, 128)],
                                        x_t[:, ts(n, 512)]))
                        for i, (lh, rh) in enumerate(mms):
                            nc.tensor.matmul(ps[:], lh, rh, start=(i == 0), stop=(i == len(mms) - 1))
                        nc.scalar.activation(xw[(b, m)][:, 2 + n * 512:2 + n * 512 + 512],
                                             ps[:], AF.Identity, bias=clbt[:, m:m + 1])
            for b in range(BL):
                for c in range(2):
                    nc.gpsimd.memset(ow[(b, c)][:], 0.0)
        for l in range(NL_P, NL_P + NL_C):
            wavenet_layer(l, last=(l == NL_P + NL_C - 1))
        branch_proj('cl2_w', 'cl2_b', br_scr)
        wn_stack.close()
        zbase([1, 2], lambda b, dt, tt: br_scr[(b, dt)][:, ts(tt, 128)])
        chain_q(1, load_cbn(1))
        chain_q(2, load_cbn(2))

        # ================= P4: residual group =================
        zbase([3, 4, 5], lambda b, dt, tt: ap('x')[b, dt * 128:(dt + 1) * 128, ts(tt, 128)])
        chain_q(3, load_cbn(3))
        chain_q(4, load_cbn(4))
        chain_q(5, load_cbn(5))

        # ================= P5: gathers, outputs, LN =================
        with tc.tile_pool(name="p5", bufs=8) as p5g, \
             tc.tile_pool(name="p5w", bufs=4) as p5w, \
             tc.tile_pool(name="p5c", bufs=1) as p5c:
            # gamma/beta broadcast tiles
            gb = {}
            for b in range(BL):
                grow = p5c.tile([1, 1024], F32, name=f"grow{b}", tag=f"grow{b}")
                brow = p5c.tile([1, 1024], F32, name=f"brow{b}", tag=f"brow{b}")
                nc.sync.dma_start(grow[:], st_scr[b][0:1, 0:1024])
                nc.sync.dma_start(brow[:], st_scr[b][0:1, 1024:2048])
                gt = p5c.tile([128, 1024], F32, name=f"gbc{b}", tag=f"gbc{b}")
                bt_ = p5c.tile([128, 1024], F32, name=f"bbc{b}", tag=f"bbc{b}")
                nc.gpsimd.partition_broadcast(gt[:], grow[:])
                nc.gpsimd.partition_broadcast(bt_[:], brow[:])
                gb[b] = (gt, bt_)

            for b in range(BL):
                for tt in range(NT):
                    g = []
                    for q in range(6):
                        gt_ = p5g.tile([128, 1032], F32, name=f"g{q}{b}{tt}", tag="bg", bufs=8)
                        nc.gpsimd.indirect_dma_start(
                            out=gt_[:], out_offset=None, in_=ap(f'big{q}'),
                            in_offset=bass.IndirectOffsetOnAxis(
                                ap=idxT[:, icol(q, b, tt):icol(q, b, tt) + 1], axis=0))
                        g.append(gt_)
                        # loss: (z_eT - cb[idx])^2 summed
                        df = p5w.tile([128, 8], F32, name=f"df{q}{b}{tt}", tag="df", bufs=2)
                        nc.vector.tensor_sub(df[:], zeT[:, ts(zcol(q, b, tt) // 8, 8)],
                                             gt_[:, 1024:1032])
                        dsq = p5w.tile([128, 8], F32, name=f"dsq{q}{b}{tt}", tag="dsq", bufs=2)
                        nc.scalar.activation(dsq[:], df[:], AF.Square,
                                             accum_out=sseT[:, icol(q, b, tt):icol(q, b, tt) + 1])
                    # outputs
                    trow = slice(tt * 128, (tt + 1) * 128)
                    nc.sync.dma_start(ap('zp_td')[b, trow, :], g[0][:, 0:1024])
                    zc_ = p5w.tile([128, 1024], F32, name=f"zc{b}{tt}", tag="zacc", bufs=6)
                    nc.vector.tensor_add(zc_[:], g[1][:, 0:1024], g[2][:, 0:1024])
                    nc.sync.dma_start(ap('zc_td')[b, trow, :], zc_[:])
                    zr_ = p5w.tile([128, 1024], F32, name=f"zr{b}{tt}", tag="zacc", bufs=6)
                    nc.vector.tensor_add(zr_[:], g[3][:, 0:1024], g[4][:, 0:1024])
                    nc.vector.tensor_add(zr_[:], zr_[:], g[5][:, 0:1024])
                    nc.sync.dma_start(ap('zr_td')[b, trow, :], zr_[:])
                    outs_ = p5w.tile([128, 1024], F32, name=f"outs{b}{tt}", tag="zacc", bufs=6)
                    nc.vector.tensor_add(outs_[:], g[0][:, 0:1024], zc_[:])
                    nc.vector.scalar_tensor_tensor(
                        out=outs_[:], in0=zr_[:], scalar=maskt[:, b:b + 1], in1=outs_[:],
                        op0=ALU.mult, op1=ALU.add)
                    # layernorm over free dim
                    s1 = p5w.tile([128, 1], F32, name=f"s1{b}{tt}", tag="ln1", bufs=12)
                    nc.vector.reduce_sum(s1[:], outs_[:], axis=mybir.AxisListType.X)
                    mu = p5w.tile([128, 1], F32, name=f"mu{b}{tt}", tag="ln1", bufs=12)
                    nc.scalar.mul(mu[:], s1[:], 1.0 / D)
                    sqd = p5w.tile([128, 1024], F32, name=f"sqd{b}{tt}", tag="sqd", bufs=2)
                    s2 = p5w.tile([128, 1], F32, name=f"s2{b}{tt}", tag="ln1", bufs=12)
                    nc.scalar.activation(sqd[:], outs_[:], AF.Square, accum_out=s2[:])
                    e2 = p5w.tile([128, 1], F32, name=f"e2{b}{tt}", tag="ln1", bufs=12)
                    nc.scalar.mul(e2[:], s2[:], 1.0 / D)
                    mu2 = p5w.tile([128, 1], F32, name=f"mu2{b}{tt}", tag="ln1", bufs=12)
                    nc.vector.tensor_mul(mu2[:], mu[:], mu[:])
                    var = p5w.tile([128, 1], F32, name=f"var{b}{tt}", tag="ln1", bufs=12)
                    nc.vector.tensor_sub(var[:], e2[:], mu2[:])
                    nc.vector.tensor_scalar(out=var[:], in0=var[:], scalar1=1e-5,
                                            scalar2=None, op0=ALU.add)
                    std = p5w.tile([128, 1], F32, name=f"std{b}{tt}", tag="ln1", bufs=12)
                    nc.scalar.activation(std[:], var[:], AF.Sqrt)
                    rstd = p5w.tile([128, 1], F32, name=f"rstd{b}{tt}", tag="ln1", bufs=12)
                    nc.vector.reciprocal(rstd[:], std[:])
                    nb = p5w.tile([128, 1], F32, name=f"nb{b}{tt}", tag="ln1", bufs=12)
                    nc.vector.tensor_mul(nb[:], mu[:], rstd[:])
                    nc.vector.tensor_scalar(out=nb[:], in0=nb[:], scalar1=-1.0,
                                            scalar2=None, op0=ALU.mult)
                    nt_ = p5w.tile([128, 1024], F32, name=f"nt{b}{tt}", tag="zacc", bufs=6)
                    nc.scalar.activation(nt_[:], outs_[:], AF.Identity,
                                         bias=nb[:], scale=rstd[:])
                    nc.vector.tensor_mul(nt_[:], nt_[:], gb[b][0][:])
                    nc.vector.tensor_add(nt_[:], nt_[:], gb[b][1][:])
                    nc.sync.dma_start(ap('o_td')[b, trow, :], nt_[:])

            # SSE reduce
            sred = p5c.tile([128, 1], F32, name="sred", tag="sred")
            nc.vector.reduce_sum(sred[:], sseT[:], axis=mybir.AxisListType.X)
            nc.sync.dma_start(ap('sse_o'), sred[:])

    nc.compile()
    return nc


_CACHE = {}


def kernel(**inputs):
    if 'nc' not in _CACHE:
        _CACHE['nc'] = build_nc()
    nc = _CACHE['nc']
    prep = host_prepare(inputs)

    x = np.ascontiguousarray(np.asarray(inputs['x'], np.float32))
    pros = np.ascontiguousarray(np.asarray(inputs['prosody_feature'], np.float32))
    mel = np.ascontiguousarray(np.asarray(inputs['mel_segments'], np.float32))
    res_mask = np.asarray(inputs['res_mask'], np.float32).reshape(B)

    in_maps = []
    for ci in range(NCORES):
        sl = slice(ci * BL, (ci + 1) * BL)
        m = dict(prep)
        m['x'] = x[sl]
        m['prosody'] = pros[sl]
        m['mel'] = mel[sl]
        m['maskp'] = np.ascontiguousarray(
            np.broadcast_to(res_mask[sl, None, None], (BL, 128, 1)).astype(np.float32))
        in_maps.append(m)

    res = run_bass_kernel_spmd(nc, in_maps, core_ids=list(range(NCORES))).results

    tr = lambda k: np.ascontiguousarray(
        np.concatenate([r[k] for r in res], 0).transpose(0, 2, 1))
    o = tr('o_td')
    z_p = tr('zp_td')
    z_c = tr('zc_td')
    z_r = tr('zr_td')
    timbre = np.concatenate([r['timbre_o'] for r in res], 0)
    sse = np.float32(sum(float(r['sse_o'].sum(dtype=np.float64)) for r in res))
    commit = np.float32(sse / np.float32(B * CB_D * T))
    return (o, z_p, z_c, z_r, commit, commit, timbre)
